# revision 7
# baseline (speedup 1.0000x reference)
"""CGCNN forward pass on 8 Trainium2 NeuronCores (Bass/Tile).

Key algebraic identity exploited: the reference uses row = edge_index[0] for
BOTH the gather (h[row]) and the scatter (segment_sum(..., row)), so

    segment_sum(h[row] * ef, row) == h * segment_sum(ef, row)

i.e. the per-edge gather of node features disappears entirely, and the edge
MLP outputs for all 4 conv layers (which depend only on edge_attr) can be
computed in a single pass with stacked weights [64, 4*128].

Sharding: nodes (and the edges that scatter into them) are partitioned into 8
contiguous ranges of 5000 nodes -> one range per core.  Every core's work is
fully independent (no collectives); the only cross-core reduction is the final
mean pool, done on host over 8 [128]-vectors, followed by the (tiny) dense
head in numpy.

Device pipeline per core (edges grouped into 128-edge tiles per 128-node
window, two tiles = one "pair"):
  - EF:    ef_psum[128e, 2*512] = eaT.T @ Wstack   (4x 64x64 tile_position-
           packed matmuls per pair when be == 0; K=65 bias-row fallback else)
  - evict: ef_sbuf = relu(ef_psum) cast bf16       (ScalarE/VectorE alternate)
  - scatter (lagged one pair): S_psum[128n, 512] += onehot.T @ ef_sbuf
           per node window; PSUM accumulation == segment_sum for free
  - per window: evict S, 4x PE-transpose to S^T in [h, n] layout
  - node stage in [h, n] layout: h = W_emb^T x^T (+bias row), then 4x
    (hs = h*S_l; u = Wn_l^T hs; u = A_l*relu(u + bn_l)+B_l; h += u), BN
    folded into per-partition affine A,B.  Mean-pool partial -> [128,1].
"""

import os
import sys

for _p in ("/opt/trn_rl_repo",):
    if _p not in sys.path and os.path.isdir(_p):
        sys.path.insert(0, _p)

import numpy as np
import ml_dtypes

import concourse.bass as bass  # noqa: F401
import concourse.mybir as mybir
import concourse.tile as tile
from concourse import bacc
from concourse.bass_utils import run_bass_kernel_spmd

N, E = 40000, 640000
NODE_F, EDGE_F, H = 92, 64, 128
L_CONV, L_DENSE = 4, 2
EPS = 1e-3

NCORES = 8
NPC = N // NCORES              # 5000 nodes per core
WIN = 128                      # node window (psum partition dim)
NWIN = (NPC + WIN - 1) // WIN  # 40
NPAD = NWIN * WIN              # 5120
HS = L_CONV * H                # 512 stacked hidden
BF16 = ml_dtypes.bfloat16

_cache = {}


def _host_prep(x, edge_index, edge_attr, W_emb, b_emb, We, be, packed):
    """Bucket+sort edges by destination node, pad to 128-edge tiles per
    128-node window (uniform tile counts across cores, NT even), build
    one-hot tiles and transposed inputs, all bf16."""
    row = np.asarray(edge_index)[0].astype(np.int64)
    ea = np.asarray(edge_attr, np.float32)

    core = row // NPC
    local = row - core * NPC
    win = local // WIN
    col = (local - win * WIN).astype(np.int32)
    key = (core * NWIN + win).astype(np.int64)

    order = np.argsort(key, kind="stable")
    counts = np.bincount(key, minlength=NCORES * NWIN).reshape(NCORES, NWIN)
    Tw = np.maximum(np.ceil(counts.max(axis=0) / 128).astype(np.int64), 1)
    if Tw.sum() % 2:
        Tw[-1] += 1  # keep NT even so tiles pair cleanly
    NT = int(Tw.sum())
    EP = NT * 128
    tile_start = np.zeros(NWIN, np.int64)
    tile_start[1:] = np.cumsum(Tw)[:-1]
    slot_start = tile_start * 128

    seg_end = np.cumsum(counts.reshape(-1))
    seg_start = seg_end - counts.reshape(-1)

    ea64 = np.zeros((NCORES, 64, EP), np.float32)
    colslot = np.full((NCORES, EP), -1, np.int32)
    for c in range(NCORES):
        for w in range(NWIN):
            k = c * NWIN + w
            idx = order[seg_start[k]:seg_end[k]]
            n = len(idx)
            if n == 0:
                continue
            s0 = slot_start[w]
            ea64[c, :, s0:s0 + n] = ea[idx].T
            colslot[c, s0:s0 + n] = col[idx]

    if packed:
        # pair layout: rows 0:64 = even tile features, 64:128 = odd tile
        a = ea64.reshape(NCORES, 64, NT, 128)
        eat = np.concatenate(
            [a[:, :, 0::2, :].reshape(NCORES, 64, EP // 2),
             a[:, :, 1::2, :].reshape(NCORES, 64, EP // 2)], axis=1)
    else:
        eat = np.concatenate(
            [ea64, np.ones((NCORES, 1, EP), np.float32)], axis=1)  # bias row

    # one-hot, layout [core, e_in_tile(partition), tile, node_in_window]
    oh = np.zeros((NCORES, 128, NT, 128), np.float32)
    cc, ss = np.nonzero(colslot >= 0)
    t_idx = ss // 128
    e_in = ss - t_idx * 128
    oh[cc, e_in, t_idx, colslot[cc, ss]] = 1.0
    oh = oh.reshape(NCORES, 128, NT * 128)

    # node features transposed + ones row, padded to NPAD
    xt = np.zeros((NCORES, 93, NPAD), np.float32)
    xf = np.asarray(x, np.float32)
    for c in range(NCORES):
        xt[c, :NODE_F, :NPC] = xf[c * NPC:(c + 1) * NPC].T
    xt[:, 92, :] = 1.0

    wemb93 = np.zeros((93, 128), np.float32)
    wemb93[:NODE_F] = np.asarray(W_emb, np.float32)
    wemb93[92] = np.asarray(b_emb, np.float32)

    Wef = np.asarray(We, np.float32)
    bef = np.asarray(be, np.float32)
    if packed:
        wstack = np.zeros((128, HS), np.float32)
        for l in range(L_CONV):
            wstack[:64, l * H:(l + 1) * H] = Wef[l]
            wstack[64:, l * H:(l + 1) * H] = Wef[l]
    else:
        wstack = np.zeros((65, HS), np.float32)
        for l in range(L_CONV):
            wstack[:64, l * H:(l + 1) * H] = Wef[l]
            wstack[64, l * H:(l + 1) * H] = bef[l]

    return {
        "NT": NT, "Tw": Tw, "EP": EP,
        "eat": np.ascontiguousarray(eat).astype(BF16),
        "oh": oh.astype(BF16),
        "xt": xt.astype(BF16),
        "wemb": wemb93.astype(BF16),
        "wstack": wstack.astype(BF16),
    }


def _build_program(NT, Tw, packed):
    from concourse.masks import make_identity

    EP = NT * 128
    f32 = mybir.dt.float32
    bf = mybir.dt.bfloat16
    Relu = mybir.ActivationFunctionType.Relu
    MULT = mybir.AluOpType.mult
    ADD = mybir.AluOpType.add

    nc = bacc.Bacc(None, target_bir_lowering=False)
    d_eat = nc.dram_tensor(
        "eat", [128 if packed else 65, EP // 2 if packed else EP], bf,
        kind="ExternalInput")
    d_oh = nc.dram_tensor("oh", [128, EP], bf, kind="ExternalInput")
    d_xt = nc.dram_tensor("xt", [93, NPAD], bf, kind="ExternalInput")
    d_wemb = nc.dram_tensor("wemb", [93, 128], bf, kind="ExternalInput")
    d_wstack = nc.dram_tensor(
        "wstack", [128 if packed else 65, HS], bf, kind="ExternalInput")
    d_wn = nc.dram_tensor("wn", [128, HS], bf, kind="ExternalInput")
    d_bnb = nc.dram_tensor("bnb", [128, L_CONV], f32, kind="ExternalInput")
    d_ab = nc.dram_tensor("ab", [128, L_CONV], f32, kind="ExternalInput")
    d_bb = nc.dram_tensor("bb", [128, L_CONV], f32, kind="ExternalInput")
    d_out = nc.dram_tensor("hsum", [128, 1], f32, kind="ExternalOutput")

    NPAIR = NT // 2
    GP = 4                      # pairs per DMA group (8 tiles)
    LAG = 3                     # scatter lags LAG pairs

    # window id per tile
    win_of = np.repeat(np.arange(NWIN), Tw)
    first_of = np.zeros(NT, bool)
    last_of = np.zeros(NT, bool)
    pos = 0
    for w in range(NWIN):
        first_of[pos] = True
        last_of[pos + int(Tw[w]) - 1] = True
        pos += int(Tw[w])

    with tile.TileContext(nc) as tc:
        with (
            tc.tile_pool(name="const", bufs=1) as constp,
            tc.tile_pool(name="ea", bufs=3) as eap,
            tc.tile_pool(name="ohp", bufs=4) as ohp,
            tc.tile_pool(name="ef", bufs=6) as efp,
            tc.tile_pool(name="swin", bufs=2) as swinp,
            tc.tile_pool(name="efps", bufs=5, space="PSUM") as efpsp,
            tc.tile_pool(name="sps", bufs=3, space="PSUM") as spsp,
            tc.tile_pool(name="big", bufs=1) as bigp,
        ):
            wstack_sb = constp.tile([128 if packed else 65, HS], bf)
            nc.sync.dma_start(wstack_sb[:], d_wstack[:])
            wemb_sb = constp.tile([93, 128], bf)
            nc.sync.dma_start(wemb_sb[:], d_wemb[:])
            wn_sb = constp.tile([128, HS], bf)
            nc.sync.dma_start(wn_sb[:], d_wn[:])
            bnb_sb = constp.tile([128, L_CONV], f32)
            nc.sync.dma_start(bnb_sb[:], d_bnb[:])
            ab_sb = constp.tile([128, L_CONV], f32)
            nc.sync.dma_start(ab_sb[:], d_ab[:])
            bb_sb = constp.tile([128, L_CONV], f32)
            nc.sync.dma_start(bb_sb[:], d_bb[:])
            ident_sb = constp.tile([128, 128], bf)
            make_identity(nc, ident_sb[:])

            xt_sb = bigp.tile([93, NPAD], bf)
            NK = NPAD // 512
            for k in range(NK):
                nc.sync.dma_start(
                    xt_sb[:, k * 512:(k + 1) * 512],
                    d_xt[:, k * 512:(k + 1) * 512])
            st_sb = bigp.tile([128, L_CONV * NPAD], bf)   # S^T per layer
            h_sb = bigp.tile([128, NPAD], bf)
            hs_sb = bigp.tile([128, NPAD], bf)
            t_sb = bigp.tile([128, NPAD], bf)
            u_sb = bigp.tile([128, NPAD], bf)
            hsum_sb = bigp.tile([128, 1], f32)

            def emit_node_chunk(k):
                """Embed + all 4 conv layers for node chunk k (512 nodes).
                Elementwise on GpSimd (idle engine), relu+bias on ScalarE,
                matmuls on TensorE.  Requires windows 4k..4k+3 transposed."""
                sl = slice(k * 512, (k + 1) * 512)
                h_ps = spsp.tile([128, 512], f32, tag="sps", name="h_ps")
                nc.tensor.matmul(
                    h_ps[:], wemb_sb[:], xt_sb[:, sl], start=True, stop=True)
                nc.scalar.copy(h_sb[:, sl], h_ps[:])
                for l in range(L_CONV):
                    nc.gpsimd.tensor_tensor(
                        hs_sb[:, sl], h_sb[:, sl],
                        st_sb[:, l * NPAD + k * 512:l * NPAD + (k + 1) * 512],
                        op=MULT)
                    u_ps = spsp.tile([128, 512], f32, tag="sps", name="u_ps")
                    nc.tensor.matmul(
                        u_ps[:], wn_sb[:, l * 128:(l + 1) * 128],
                        hs_sb[:, sl], start=True, stop=True)
                    nc.scalar.activation(
                        t_sb[:, sl], u_ps[:], Relu,
                        bias=bnb_sb[:, l:l + 1], scale=1.0)
                    nc.gpsimd.tensor_scalar(
                        u_sb[:, sl], t_sb[:, sl], ab_sb[:, l:l + 1],
                        bb_sb[:, l:l + 1], op0=MULT, op1=ADD)
                    nc.gpsimd.tensor_tensor(
                        h_sb[:, sl], h_sb[:, sl], u_sb[:, sl], op=ADD)

            # ---------------- edge stage (software-pipelined) ------------
            state = {"s_ps": None}
            ef_tiles = [None] * NPAIR
            oh_groups = [None] * NPAIR

            def emit_scatter(t, ef_pair, oh_g, goff):
                w = int(win_of[t])
                if first_of[t]:
                    state["s_ps"] = spsp.tile(
                        [128, HS], f32, tag="sps", name="s_ps")
                s_ps = state["s_ps"]
                j = t - goff
                nc.tensor.matmul(
                    s_ps[:], oh_g[:, j * 128:(j + 1) * 128],
                    ef_pair[:, (t % 2) * HS:(t % 2 + 1) * HS],
                    start=bool(first_of[t]), stop=bool(last_of[t]))
                if last_of[t]:
                    s_sb = swinp.tile([128, HS], bf, tag="swin")
                    nc.scalar.copy(s_sb[:], s_ps[:])
                    for l in range(L_CONV):
                        tr_ps = spsp.tile([128, 128], bf, tag="sps")
                        nc.tensor.transpose(
                            tr_ps[:], s_sb[:, l * 128:(l + 1) * 128],
                            ident_sb[:])
                        dst = st_sb[:, l * NPAD + w * WIN:
                                    l * NPAD + (w + 1) * WIN]
                        nc.vector.tensor_copy(dst, tr_ps[:])
                    if (w + 1) % 4 == 0:
                        emit_node_chunk((w + 1) // 4 - 1)

            parity = 0
            ea_g = oh_g = None
            g0p = 0
            for p in range(NPAIR):
                if p % GP == 0:
                    gs = min(GP, NPAIR - p)
                    g0p = p
                    ea_g = eap.tile(
                        [128, GP * 128] if packed else [65, GP * 256], bf,
                        tag="ea")
                    if packed:
                        nc.sync.dma_start(
                            ea_g[:, :gs * 128],
                            d_eat[:, p * 128:(p + gs) * 128])
                    else:
                        nc.sync.dma_start(
                            ea_g[:, :gs * 256],
                            d_eat[:, p * 256:(p + gs) * 256])
                    oh_g = ohp.tile([128, GP * 256], bf, tag="oh")
                    nc.sync.dma_start(
                        oh_g[:, :gs * 256], d_oh[:, p * 256:(p + gs) * 256])

                jp = p - g0p
                ef_ps0 = efpsp.tile([128, HS], f32, tag="efps", name="ef_ps0")
                ef_ps1 = efpsp.tile([128, HS], f32, tag="efps", name="ef_ps1")
                if packed:
                    pe = ea_g[:, jp * 128:(jp + 1) * 128]
                    nc.tensor.matmul(
                        ef_ps0[0:64, :], pe[0:64, 0:64], wstack_sb[0:64, :],
                        start=True, stop=True, tile_position=(0, 0))
                    nc.tensor.matmul(
                        ef_ps0[64:128, :], pe[0:64, 64:128],
                        wstack_sb[0:64, :],
                        start=True, stop=True, tile_position=(0, 64))
                    nc.tensor.matmul(
                        ef_ps1[0:64, :], pe[64:128, 0:64],
                        wstack_sb[64:128, :],
                        start=True, stop=True, tile_position=(64, 0))
                    nc.tensor.matmul(
                        ef_ps1[64:128, :], pe[64:128, 64:128],
                        wstack_sb[64:128, :],
                        start=True, stop=True, tile_position=(64, 64))
                else:
                    nc.tensor.matmul(
                        ef_ps0[:], ea_g[:, jp * 256:jp * 256 + 128],
                        wstack_sb[:], start=True, stop=True)
                    nc.tensor.matmul(
                        ef_ps1[:],
                        ea_g[:, jp * 256 + 128:(jp + 1) * 256],
                        wstack_sb[:], start=True, stop=True)

                ef_pair = efp.tile([128, 2 * HS], bf, tag="ef")
                for half, ps in ((0, ef_ps0), (1, ef_ps1)):
                    dst = ef_pair[:, half * HS:(half + 1) * HS]
                    if parity:
                        nc.scalar.activation(dst, ps[:], Relu)
                    else:
                        nc.vector.tensor_scalar_max(dst, ps[:], 0.0)
                    parity ^= 1
                ef_tiles[p] = ef_pair
                oh_groups[p] = (oh_g, 2 * g0p)

                if p >= LAG:
                    q = p - LAG
                    og, goff = oh_groups[q]
                    emit_scatter(2 * q, ef_tiles[q], og, goff)
                    emit_scatter(2 * q + 1, ef_tiles[q], og, goff)
                    ef_tiles[q] = None
            for q in range(max(NPAIR - LAG, 0), NPAIR):
                og, goff = oh_groups[q]
                emit_scatter(2 * q, ef_tiles[q], og, goff)
                emit_scatter(2 * q + 1, ef_tiles[q], og, goff)

            nc.vector.tensor_reduce(
                hsum_sb[:], h_sb[:, :NPC], axis=mybir.AxisListType.X,
                op=ADD)
            nc.sync.dma_start(d_out[:], hsum_sb[:])

    nc.finalize()
    return nc


LAST_EXEC_NS = None


def kernel(x, edge_index, edge_attr, W_emb, b_emb, We, be, Wn, bn,
           g_c, beta_c, mu_c, var_c, Wd, bd, g_d, beta_d, mu_d, var_d, Wf, bf):
    global LAST_EXEC_NS

    packed = bool(np.all(np.asarray(be) == 0.0))
    prep = _host_prep(x, edge_index, edge_attr, W_emb, b_emb, We, be, packed)
    NT, Tw = prep["NT"], prep["Tw"]

    Wnf = np.asarray(Wn, np.float32)
    wn_stack = np.zeros((128, HS), np.float32)
    for l in range(L_CONV):
        wn_stack[:, l * H:(l + 1) * H] = Wnf[l]
    A = (np.asarray(g_c, np.float32)
         / np.sqrt(np.asarray(var_c, np.float32) + EPS))        # [L, H]
    B = np.asarray(beta_c, np.float32) - np.asarray(mu_c, np.float32) * A

    key = (NT, tuple(int(v) for v in Tw), packed)
    if key not in _cache:
        _cache[key] = _build_program(NT, Tw, packed)
    nc = _cache[key]

    common = {
        "wemb": prep["wemb"],
        "wstack": prep["wstack"],
        "wn": wn_stack.astype(BF16),
        "bnb": np.ascontiguousarray(np.asarray(bn, np.float32).T).reshape(128, L_CONV),
        "ab": np.ascontiguousarray(A.T).reshape(128, L_CONV),
        "bb": np.ascontiguousarray(B.T).reshape(128, L_CONV),
    }
    in_maps = []
    for c in range(NCORES):
        m = dict(common)
        m["eat"] = prep["eat"][c]
        m["oh"] = prep["oh"][c]
        m["xt"] = prep["xt"][c]
        in_maps.append(m)

    trace = bool(os.environ.get("KERNEL_TRACE"))
    if trace:
        try:
            from trn_agent_boot.trn_boot import _ntff_profile_via_ctypes
            from antenv.axon_hooks import set_axon_ntff_profile_hook
            set_axon_ntff_profile_hook(
                _ntff_profile_via_ctypes("/opt/axon/libaxon_pjrt.so"))
        except Exception:
            trace = False

    res = run_bass_kernel_spmd(
        nc, in_maps, core_ids=list(range(NCORES)), trace=trace)
    LAST_EXEC_NS = res.exec_time_ns

    total = np.zeros(128, np.float64)
    for c in range(NCORES):
        total += res.results[c]["hsum"].reshape(128).astype(np.float64)
    v = (total / N).astype(np.float32)

    # dense head on host (0.000001% of total FLOPs)
    g_df = np.asarray(g_d, np.float32)
    var_df = np.asarray(var_d, np.float32)
    beta_df = np.asarray(beta_d, np.float32)
    mu_df = np.asarray(mu_d, np.float32)
    Wdf = np.asarray(Wd, np.float32)
    bdf = np.asarray(bd, np.float32)
    for d in range(L_DENSE):
        v = np.maximum(v @ Wdf[d] + bdf[d], 0.0)
        Ad = g_df[d] / np.sqrt(var_df[d] + EPS)
        v = (v - mu_df[d]) * Ad + beta_df[d]
    out = v @ np.asarray(Wf, np.float32) + np.asarray(bf, np.float32)
    return out.astype(np.float32)


# revision 8
# speedup vs baseline: 1.0835x; 1.0835x over previous
"""CGCNN forward pass on 8 Trainium2 NeuronCores (Bass/Tile).

Key algebraic identity exploited: the reference uses row = edge_index[0] for
BOTH the gather (h[row]) and the scatter (segment_sum(..., row)), so

    segment_sum(h[row] * ef, row) == h * segment_sum(ef, row)

i.e. the per-edge gather of node features disappears entirely, and the edge
MLP outputs for all 4 conv layers (which depend only on edge_attr) can be
computed in a single pass with stacked weights [64, 4*128].

Sharding: nodes (and the edges that scatter into them) are partitioned into 8
contiguous ranges of 5000 nodes -> one range per core.  Every core's work is
fully independent (no collectives); the only cross-core reduction is the final
mean pool, done on host over 8 [128]-vectors, followed by the (tiny) dense
head in numpy.

Device pipeline per core (edges grouped into 128-edge tiles per 128-node
window, two tiles = one "pair"):
  - EF:    ef_psum[128e, 2*512] = eaT.T @ Wstack   (4x 64x64 tile_position-
           packed matmuls per pair when be == 0; K=65 bias-row fallback else)
  - evict: ef_sbuf = relu(ef_psum) cast bf16       (ScalarE/VectorE alternate)
  - scatter (lagged one pair): S_psum[128n, 512] += onehot.T @ ef_sbuf
           per node window; PSUM accumulation == segment_sum for free
  - per window: evict S, 4x PE-transpose to S^T in [h, n] layout
  - node stage in [h, n] layout: h = W_emb^T x^T (+bias row), then 4x
    (hs = h*S_l; u = Wn_l^T hs; u = A_l*relu(u + bn_l)+B_l; h += u), BN
    folded into per-partition affine A,B.  Mean-pool partial -> [128,1].
"""

import os
import sys

for _p in ("/opt/trn_rl_repo",):
    if _p not in sys.path and os.path.isdir(_p):
        sys.path.insert(0, _p)

import numpy as np
import ml_dtypes

import concourse.bass as bass  # noqa: F401
import concourse.mybir as mybir
import concourse.tile as tile
from concourse import bacc
from concourse.bass_utils import run_bass_kernel_spmd

N, E = 40000, 640000
NODE_F, EDGE_F, H = 92, 64, 128
L_CONV, L_DENSE = 4, 2
EPS = 1e-3

NCORES = 8
NPC = N // NCORES              # 5000 nodes per core
WIN = 128                      # node window (psum partition dim)
NWIN = (NPC + WIN - 1) // WIN  # 40
NPAD = NWIN * WIN              # 5120
HS = L_CONV * H                # 512 stacked hidden
BF16 = ml_dtypes.bfloat16

_cache = {}


def _host_prep(x, edge_index, edge_attr, W_emb, b_emb, We, be, packed):
    """Bucket+sort edges by destination node, pad to 128-edge tiles per
    128-node window (uniform tile counts across cores, NT even), build
    one-hot tiles and transposed inputs, all bf16."""
    row = np.asarray(edge_index)[0].astype(np.int64)
    ea = np.asarray(edge_attr, np.float32)

    core = row // NPC
    local = row - core * NPC
    win = local // WIN
    col = (local - win * WIN).astype(np.int32)
    key = (core * NWIN + win).astype(np.int64)

    order = np.argsort(key, kind="stable")
    counts = np.bincount(key, minlength=NCORES * NWIN).reshape(NCORES, NWIN)
    Tw = np.maximum(np.ceil(counts.max(axis=0) / 128).astype(np.int64), 1)
    if Tw.sum() % 2:
        Tw[-1] += 1  # keep NT even so tiles pair cleanly
    NT = int(Tw.sum())
    EP = NT * 128
    tile_start = np.zeros(NWIN, np.int64)
    tile_start[1:] = np.cumsum(Tw)[:-1]
    slot_start = tile_start * 128

    seg_end = np.cumsum(counts.reshape(-1))
    seg_start = seg_end - counts.reshape(-1)

    ea64 = np.zeros((NCORES, 64, EP), np.float32)
    colslot = np.full((NCORES, EP), -1, np.int32)
    for c in range(NCORES):
        for w in range(NWIN):
            k = c * NWIN + w
            idx = order[seg_start[k]:seg_end[k]]
            n = len(idx)
            if n == 0:
                continue
            s0 = slot_start[w]
            ea64[c, :, s0:s0 + n] = ea[idx].T
            colslot[c, s0:s0 + n] = col[idx]

    if packed:
        # pair layout: rows 0:64 = even tile features, 64:128 = odd tile
        a = ea64.reshape(NCORES, 64, NT, 128)
        eat = np.concatenate(
            [a[:, :, 0::2, :].reshape(NCORES, 64, EP // 2),
             a[:, :, 1::2, :].reshape(NCORES, 64, EP // 2)], axis=1)
    else:
        eat = np.concatenate(
            [ea64, np.ones((NCORES, 1, EP), np.float32)], axis=1)  # bias row

    # one-hot, layout [core, e_in_tile(partition), tile, node_in_window]
    oh = np.zeros((NCORES, 128, NT, 128), np.float32)
    cc, ss = np.nonzero(colslot >= 0)
    t_idx = ss // 128
    e_in = ss - t_idx * 128
    oh[cc, e_in, t_idx, colslot[cc, ss]] = 1.0
    oh = oh.reshape(NCORES, 128, NT * 128)

    # node features transposed + ones row, padded to NPAD
    xt = np.zeros((NCORES, 93, NPAD), np.float32)
    xf = np.asarray(x, np.float32)
    for c in range(NCORES):
        xt[c, :NODE_F, :NPC] = xf[c * NPC:(c + 1) * NPC].T
    xt[:, 92, :] = 1.0

    wemb93 = np.zeros((93, 128), np.float32)
    wemb93[:NODE_F] = np.asarray(W_emb, np.float32)
    wemb93[92] = np.asarray(b_emb, np.float32)

    Wef = np.asarray(We, np.float32)
    bef = np.asarray(be, np.float32)
    if packed:
        wstack = np.zeros((128, HS), np.float32)
        for l in range(L_CONV):
            wstack[:64, l * H:(l + 1) * H] = Wef[l]
            wstack[64:, l * H:(l + 1) * H] = Wef[l]
    else:
        wstack = np.zeros((65, HS), np.float32)
        for l in range(L_CONV):
            wstack[:64, l * H:(l + 1) * H] = Wef[l]
            wstack[64, l * H:(l + 1) * H] = bef[l]

    return {
        "NT": NT, "Tw": Tw, "EP": EP,
        "eat": np.ascontiguousarray(eat).astype(BF16),
        "oh": oh.astype(BF16),
        "xt": xt.astype(BF16),
        "wemb": wemb93.astype(BF16),
        "wstack": wstack.astype(BF16),
    }


def _build_program(NT, Tw, packed):
    from concourse.masks import make_identity

    EP = NT * 128
    f32 = mybir.dt.float32
    bf = mybir.dt.bfloat16
    Relu = mybir.ActivationFunctionType.Relu
    MULT = mybir.AluOpType.mult
    ADD = mybir.AluOpType.add

    nc = bacc.Bacc(None, target_bir_lowering=False)
    d_eat = nc.dram_tensor(
        "eat", [128 if packed else 65, EP // 2 if packed else EP], bf,
        kind="ExternalInput")
    d_oh = nc.dram_tensor("oh", [128, EP], bf, kind="ExternalInput")
    d_xt = nc.dram_tensor("xt", [93, NPAD], bf, kind="ExternalInput")
    d_wemb = nc.dram_tensor("wemb", [93, 128], bf, kind="ExternalInput")
    d_wstack = nc.dram_tensor(
        "wstack", [128 if packed else 65, HS], bf, kind="ExternalInput")
    d_wn = nc.dram_tensor("wn", [128, HS], bf, kind="ExternalInput")
    d_bnb = nc.dram_tensor("bnb", [128, L_CONV], f32, kind="ExternalInput")
    d_ab = nc.dram_tensor("ab", [128, L_CONV], f32, kind="ExternalInput")
    d_bb = nc.dram_tensor("bb", [128, L_CONV], f32, kind="ExternalInput")
    d_out = nc.dram_tensor("hsum", [128, 1], f32, kind="ExternalOutput")

    NPAIR = NT // 2
    GP = 4                      # pairs per DMA group (8 tiles)
    LAG = 3                     # scatter lags LAG pairs

    # window id per tile
    win_of = np.repeat(np.arange(NWIN), Tw)
    first_of = np.zeros(NT, bool)
    last_of = np.zeros(NT, bool)
    pos = 0
    for w in range(NWIN):
        first_of[pos] = True
        last_of[pos + int(Tw[w]) - 1] = True
        pos += int(Tw[w])

    with tile.TileContext(nc) as tc:
        with (
            tc.tile_pool(name="const", bufs=1) as constp,
            tc.tile_pool(name="ea", bufs=3) as eap,
            tc.tile_pool(name="ohp", bufs=4) as ohp,
            tc.tile_pool(name="ef", bufs=6) as efp,
            tc.tile_pool(name="swin", bufs=2) as swinp,
            tc.tile_pool(name="efps", bufs=5, space="PSUM") as efpsp,
            tc.tile_pool(name="sps", bufs=2, space="PSUM") as spsp,
            tc.tile_pool(name="big", bufs=1) as bigp,
        ):
            wstack_sb = constp.tile([128 if packed else 65, HS], bf)
            nc.sync.dma_start(wstack_sb[:], d_wstack[:])
            wemb_sb = constp.tile([93, 128], bf)
            nc.sync.dma_start(wemb_sb[:], d_wemb[:])
            wn_sb = constp.tile([128, HS], bf)
            nc.sync.dma_start(wn_sb[:], d_wn[:])
            bnb_sb = constp.tile([128, L_CONV], f32)
            nc.sync.dma_start(bnb_sb[:], d_bnb[:])
            ab_sb = constp.tile([128, L_CONV], f32)
            nc.sync.dma_start(ab_sb[:], d_ab[:])
            bb_sb = constp.tile([128, L_CONV], f32)
            nc.sync.dma_start(bb_sb[:], d_bb[:])
            ident_sb = constp.tile([128, 128], bf)
            make_identity(nc, ident_sb[:])

            xt_sb = bigp.tile([93, NPAD], bf)
            NK = NPAD // 512
            for k in range(NK):
                nc.sync.dma_start(
                    xt_sb[:, k * 512:(k + 1) * 512],
                    d_xt[:, k * 512:(k + 1) * 512])
            st_sb = bigp.tile([128, L_CONV * NPAD], bf)   # S^T per layer
            h_sb = bigp.tile([128, NPAD], bf)
            hs_sb = bigp.tile([128, NPAD], bf)
            t_sb = bigp.tile([128, NPAD], bf)
            u_sb = bigp.tile([128, NPAD], bf)
            hsum_sb = bigp.tile([128, 1], f32)

            def emit_node_chunk(k):
                """Embed + all 4 conv layers for node chunk k (512 nodes).
                Elementwise on GpSimd (idle engine), relu+bias on ScalarE,
                matmuls on TensorE.  Requires windows 4k..4k+3 transposed."""
                sl = slice(k * 512, (k + 1) * 512)
                h_ps = spsp.tile([128, 512], f32, tag="nodeps", bufs=1, name="h_ps")
                nc.tensor.matmul(
                    h_ps[:], wemb_sb[:], xt_sb[:, sl], start=True, stop=True)
                nc.scalar.copy(h_sb[:, sl], h_ps[:])
                for l in range(L_CONV):
                    nc.gpsimd.tensor_tensor(
                        hs_sb[:, sl], h_sb[:, sl],
                        st_sb[:, l * NPAD + k * 512:l * NPAD + (k + 1) * 512],
                        op=MULT)
                    u_ps = spsp.tile([128, 512], f32, tag="nodeps", bufs=1, name="u_ps")
                    nc.tensor.matmul(
                        u_ps[:], wn_sb[:, l * 128:(l + 1) * 128],
                        hs_sb[:, sl], start=True, stop=True)
                    nc.scalar.activation(
                        t_sb[:, sl], u_ps[:], Relu,
                        bias=bnb_sb[:, l:l + 1], scale=1.0)
                    nc.gpsimd.tensor_scalar(
                        u_sb[:, sl], t_sb[:, sl], ab_sb[:, l:l + 1],
                        bb_sb[:, l:l + 1], op0=MULT, op1=ADD)
                    nc.gpsimd.tensor_tensor(
                        h_sb[:, sl], h_sb[:, sl], u_sb[:, sl], op=ADD)

            # ---------------- edge stage (software-pipelined) ------------
            state = {"s_ps": None}
            ef_tiles = [None] * NPAIR
            oh_groups = [None] * NPAIR

            def emit_scatter(t, ef_pair, oh_g, goff):
                w = int(win_of[t])
                if first_of[t]:
                    state["s_ps"] = spsp.tile(
                        [128, HS], f32, tag="sps", name="s_ps")
                s_ps = state["s_ps"]
                j = t - goff
                nc.tensor.matmul(
                    s_ps[:], oh_g[:, j * 128:(j + 1) * 128],
                    ef_pair[:, (t % 2) * HS:(t % 2 + 1) * HS],
                    start=bool(first_of[t]), stop=bool(last_of[t]))
                if last_of[t]:
                    s_sb = swinp.tile([128, HS], bf, tag="swin")
                    nc.scalar.copy(s_sb[:], s_ps[:])
                    for l in range(L_CONV):
                        tr_ps = spsp.tile([128, 128], bf, tag="sps")
                        nc.tensor.transpose(
                            tr_ps[:], s_sb[:, l * 128:(l + 1) * 128],
                            ident_sb[:])
                        dst = st_sb[:, l * NPAD + w * WIN:
                                    l * NPAD + (w + 1) * WIN]
                        nc.vector.tensor_copy(dst, tr_ps[:])
                    if (w + 1) % 4 == 0:
                        emit_node_chunk((w + 1) // 4 - 1)

            parity = 0
            ea_g = oh_g = None
            g0p = 0
            for p in range(NPAIR):
                if p % GP == 0:
                    gs = min(GP, NPAIR - p)
                    g0p = p
                    ea_g = eap.tile(
                        [128, GP * 128] if packed else [65, GP * 256], bf,
                        tag="ea")
                    if packed:
                        nc.sync.dma_start(
                            ea_g[:, :gs * 128],
                            d_eat[:, p * 128:(p + gs) * 128])
                    else:
                        nc.sync.dma_start(
                            ea_g[:, :gs * 256],
                            d_eat[:, p * 256:(p + gs) * 256])
                    oh_g = ohp.tile([128, GP * 256], bf, tag="oh")
                    nc.sync.dma_start(
                        oh_g[:, :gs * 256], d_oh[:, p * 256:(p + gs) * 256])

                jp = p - g0p
                ef_ps0 = efpsp.tile([128, HS], f32, tag="efps", name="ef_ps0")
                ef_ps1 = efpsp.tile([128, HS], f32, tag="efps", name="ef_ps1")
                if packed:
                    pe = ea_g[:, jp * 128:(jp + 1) * 128]
                    nc.tensor.matmul(
                        ef_ps0[0:64, :], pe[0:64, 0:64], wstack_sb[0:64, :],
                        start=True, stop=True, tile_position=(0, 0))
                    nc.tensor.matmul(
                        ef_ps0[64:128, :], pe[0:64, 64:128],
                        wstack_sb[0:64, :],
                        start=True, stop=True, tile_position=(0, 64))
                    nc.tensor.matmul(
                        ef_ps1[0:64, :], pe[64:128, 0:64],
                        wstack_sb[64:128, :],
                        start=True, stop=True, tile_position=(64, 0))
                    nc.tensor.matmul(
                        ef_ps1[64:128, :], pe[64:128, 64:128],
                        wstack_sb[64:128, :],
                        start=True, stop=True, tile_position=(64, 64))
                else:
                    nc.tensor.matmul(
                        ef_ps0[:], ea_g[:, jp * 256:jp * 256 + 128],
                        wstack_sb[:], start=True, stop=True)
                    nc.tensor.matmul(
                        ef_ps1[:],
                        ea_g[:, jp * 256 + 128:(jp + 1) * 256],
                        wstack_sb[:], start=True, stop=True)

                ef_pair = efp.tile([128, 2 * HS], bf, tag="ef")
                for half, ps in ((0, ef_ps0), (1, ef_ps1)):
                    dst = ef_pair[:, half * HS:(half + 1) * HS]
                    if parity:
                        nc.scalar.activation(dst, ps[:], Relu)
                    else:
                        nc.vector.tensor_scalar_max(dst, ps[:], 0.0)
                    parity ^= 1
                ef_tiles[p] = ef_pair
                oh_groups[p] = (oh_g, 2 * g0p)

                if p >= LAG:
                    q = p - LAG
                    og, goff = oh_groups[q]
                    emit_scatter(2 * q, ef_tiles[q], og, goff)
                    emit_scatter(2 * q + 1, ef_tiles[q], og, goff)
                    ef_tiles[q] = None
            for q in range(max(NPAIR - LAG, 0), NPAIR):
                og, goff = oh_groups[q]
                emit_scatter(2 * q, ef_tiles[q], og, goff)
                emit_scatter(2 * q + 1, ef_tiles[q], og, goff)

            nc.vector.tensor_reduce(
                hsum_sb[:], h_sb[:, :NPC], axis=mybir.AxisListType.X,
                op=ADD)
            nc.sync.dma_start(d_out[:], hsum_sb[:])

    nc.finalize()
    return nc


LAST_EXEC_NS = None


def kernel(x, edge_index, edge_attr, W_emb, b_emb, We, be, Wn, bn,
           g_c, beta_c, mu_c, var_c, Wd, bd, g_d, beta_d, mu_d, var_d, Wf, bf):
    global LAST_EXEC_NS

    packed = bool(np.all(np.asarray(be) == 0.0))
    prep = _host_prep(x, edge_index, edge_attr, W_emb, b_emb, We, be, packed)
    NT, Tw = prep["NT"], prep["Tw"]

    Wnf = np.asarray(Wn, np.float32)
    wn_stack = np.zeros((128, HS), np.float32)
    for l in range(L_CONV):
        wn_stack[:, l * H:(l + 1) * H] = Wnf[l]
    A = (np.asarray(g_c, np.float32)
         / np.sqrt(np.asarray(var_c, np.float32) + EPS))        # [L, H]
    B = np.asarray(beta_c, np.float32) - np.asarray(mu_c, np.float32) * A

    key = (NT, tuple(int(v) for v in Tw), packed)
    if key not in _cache:
        _cache[key] = _build_program(NT, Tw, packed)
    nc = _cache[key]

    common = {
        "wemb": prep["wemb"],
        "wstack": prep["wstack"],
        "wn": wn_stack.astype(BF16),
        "bnb": np.ascontiguousarray(np.asarray(bn, np.float32).T).reshape(128, L_CONV),
        "ab": np.ascontiguousarray(A.T).reshape(128, L_CONV),
        "bb": np.ascontiguousarray(B.T).reshape(128, L_CONV),
    }
    in_maps = []
    for c in range(NCORES):
        m = dict(common)
        m["eat"] = prep["eat"][c]
        m["oh"] = prep["oh"][c]
        m["xt"] = prep["xt"][c]
        in_maps.append(m)

    trace = bool(os.environ.get("KERNEL_TRACE"))
    if trace:
        try:
            from trn_agent_boot.trn_boot import _ntff_profile_via_ctypes
            from antenv.axon_hooks import set_axon_ntff_profile_hook
            set_axon_ntff_profile_hook(
                _ntff_profile_via_ctypes("/opt/axon/libaxon_pjrt.so"))
        except Exception:
            trace = False

    res = run_bass_kernel_spmd(
        nc, in_maps, core_ids=list(range(NCORES)), trace=trace)
    LAST_EXEC_NS = res.exec_time_ns

    total = np.zeros(128, np.float64)
    for c in range(NCORES):
        total += res.results[c]["hsum"].reshape(128).astype(np.float64)
    v = (total / N).astype(np.float32)

    # dense head on host (0.000001% of total FLOPs)
    g_df = np.asarray(g_d, np.float32)
    var_df = np.asarray(var_d, np.float32)
    beta_df = np.asarray(beta_d, np.float32)
    mu_df = np.asarray(mu_d, np.float32)
    Wdf = np.asarray(Wd, np.float32)
    bdf = np.asarray(bd, np.float32)
    for d in range(L_DENSE):
        v = np.maximum(v @ Wdf[d] + bdf[d], 0.0)
        Ad = g_df[d] / np.sqrt(var_df[d] + EPS)
        v = (v - mu_df[d]) * Ad + beta_df[d]
    out = v @ np.asarray(Wf, np.float32) + np.asarray(bf, np.float32)
    return out.astype(np.float32)


# revision 9
# speedup vs baseline: 1.2321x; 1.1371x over previous
"""CGCNN forward pass on 8 Trainium2 NeuronCores (Bass/Tile).

Key algebraic identity exploited: the reference uses row = edge_index[0] for
BOTH the gather (h[row]) and the scatter (segment_sum(..., row)), so

    segment_sum(h[row] * ef, row) == h * segment_sum(ef, row)

i.e. the per-edge gather of node features disappears entirely, and the edge
MLP outputs for all 4 conv layers (which depend only on edge_attr) can be
computed in a single pass with stacked weights [64, 4*128].

Sharding: nodes (and the edges that scatter into them) are partitioned into 8
contiguous ranges of 5000 nodes -> one range per core.  Every core's work is
fully independent (no collectives); the only cross-core reduction is the final
mean pool, done on host over 8 [128]-vectors, followed by the (tiny) dense
head in numpy.

Device pipeline per core (edges grouped into 128-edge tiles per 128-node
window, two tiles = one "pair"):
  - EF:    ef_psum[128e, 2*512] = eaT.T @ Wstack   (4x 64x64 tile_position-
           packed matmuls per pair when be == 0; K=65 bias-row fallback else)
  - evict: ef_sbuf = relu(ef_psum) cast bf16       (ScalarE/VectorE alternate)
  - scatter (lagged one pair): S_psum[128n, 512] += onehot.T @ ef_sbuf
           per node window; PSUM accumulation == segment_sum for free
  - per window: evict S, 4x PE-transpose to S^T in [h, n] layout
  - node stage in [h, n] layout: h = W_emb^T x^T (+bias row), then 4x
    (hs = h*S_l; u = Wn_l^T hs; u = A_l*relu(u + bn_l)+B_l; h += u), BN
    folded into per-partition affine A,B.  Mean-pool partial -> [128,1].
"""

import os
import sys

for _p in ("/opt/trn_rl_repo",):
    if _p not in sys.path and os.path.isdir(_p):
        sys.path.insert(0, _p)

import numpy as np
import ml_dtypes

import concourse.bass as bass  # noqa: F401
import concourse.mybir as mybir
import concourse.tile as tile
from concourse import bacc
from concourse.bass_utils import run_bass_kernel_spmd

N, E = 40000, 640000
NODE_F, EDGE_F, H = 92, 64, 128
L_CONV, L_DENSE = 4, 2
EPS = 1e-3

NCORES = 8
NPC = N // NCORES              # 5000 nodes per core
WIN = 128                      # node window (psum partition dim)
NWIN = (NPC + WIN - 1) // WIN  # 40
NPAD = NWIN * WIN              # 5120
HS = L_CONV * H                # 512 stacked hidden
BF16 = ml_dtypes.bfloat16

_cache = {}


def _host_prep(x, edge_index, edge_attr, W_emb, b_emb, We, be, packed):
    """Bucket+sort edges by destination node, pad to 128-edge tiles per
    128-node window (uniform tile counts across cores, NT even), build
    one-hot tiles and transposed inputs, all bf16."""
    row = np.asarray(edge_index)[0].astype(np.int64)
    ea = np.asarray(edge_attr, np.float32)

    core = row // NPC
    local = row - core * NPC
    win = local // WIN
    col = (local - win * WIN).astype(np.int32)
    key = (core * NWIN + win).astype(np.int64)

    order = np.argsort(key, kind="stable")
    counts = np.bincount(key, minlength=NCORES * NWIN).reshape(NCORES, NWIN)
    Tw = np.maximum(np.ceil(counts.max(axis=0) / 128).astype(np.int64), 1)
    if Tw.sum() % 2:
        Tw[-1] += 1  # keep NT even so tiles pair cleanly
    NT = int(Tw.sum())
    EP = NT * 128
    tile_start = np.zeros(NWIN, np.int64)
    tile_start[1:] = np.cumsum(Tw)[:-1]
    slot_start = tile_start * 128

    seg_end = np.cumsum(counts.reshape(-1))
    seg_start = seg_end - counts.reshape(-1)

    ea64 = np.zeros((NCORES, 64, EP), np.float32)
    colslot = np.full((NCORES, EP), -1, np.int32)
    for c in range(NCORES):
        for w in range(NWIN):
            k = c * NWIN + w
            idx = order[seg_start[k]:seg_end[k]]
            n = len(idx)
            if n == 0:
                continue
            s0 = slot_start[w]
            ea64[c, :, s0:s0 + n] = ea[idx].T
            colslot[c, s0:s0 + n] = col[idx]

    if packed:
        # pair layout: rows 0:64 = even tile features, 64:128 = odd tile
        a = ea64.reshape(NCORES, 64, NT, 128)
        eat = np.concatenate(
            [a[:, :, 0::2, :].reshape(NCORES, 64, EP // 2),
             a[:, :, 1::2, :].reshape(NCORES, 64, EP // 2)], axis=1)
    else:
        eat = np.concatenate(
            [ea64, np.ones((NCORES, 1, EP), np.float32)], axis=1)  # bias row

    # one-hot, layout [core, e_in_tile(partition), tile, node_in_window]
    oh = np.zeros((NCORES, 128, NT, 128), np.float32)
    cc, ss = np.nonzero(colslot >= 0)
    t_idx = ss // 128
    e_in = ss - t_idx * 128
    oh[cc, e_in, t_idx, colslot[cc, ss]] = 1.0
    oh = oh.reshape(NCORES, 128, NT * 128)

    # node features transposed + ones row, padded to NPAD
    xt = np.zeros((NCORES, 93, NPAD), np.float32)
    xf = np.asarray(x, np.float32)
    for c in range(NCORES):
        xt[c, :NODE_F, :NPC] = xf[c * NPC:(c + 1) * NPC].T
    xt[:, 92, :] = 1.0

    wemb93 = np.zeros((93, 128), np.float32)
    wemb93[:NODE_F] = np.asarray(W_emb, np.float32)
    wemb93[92] = np.asarray(b_emb, np.float32)

    Wef = np.asarray(We, np.float32)
    bef = np.asarray(be, np.float32)
    if packed:
        wstack = np.zeros((128, HS), np.float32)
        for l in range(L_CONV):
            wstack[:64, l * H:(l + 1) * H] = Wef[l]
            wstack[64:, l * H:(l + 1) * H] = Wef[l]
    else:
        wstack = np.zeros((65, HS), np.float32)
        for l in range(L_CONV):
            wstack[:64, l * H:(l + 1) * H] = Wef[l]
            wstack[64, l * H:(l + 1) * H] = bef[l]

    return {
        "NT": NT, "Tw": Tw, "EP": EP,
        "eat": np.ascontiguousarray(eat).astype(BF16),
        "oh": oh.astype(BF16),
        "xt": xt.astype(BF16),
        "wemb": wemb93.astype(BF16),
        "wstack": wstack.astype(BF16),
    }


def _build_program(NT, Tw, packed):
    from concourse.masks import make_identity

    EP = NT * 128
    f32 = mybir.dt.float32
    bf = mybir.dt.bfloat16
    Relu = mybir.ActivationFunctionType.Relu
    MULT = mybir.AluOpType.mult
    ADD = mybir.AluOpType.add

    nc = bacc.Bacc(None, target_bir_lowering=False)
    d_eat = nc.dram_tensor(
        "eat", [128 if packed else 65, EP // 2 if packed else EP], bf,
        kind="ExternalInput")
    d_oh = nc.dram_tensor("oh", [128, EP], bf, kind="ExternalInput")
    d_xt = nc.dram_tensor("xt", [93, NPAD], bf, kind="ExternalInput")
    d_wemb = nc.dram_tensor("wemb", [93, 128], bf, kind="ExternalInput")
    d_wstack = nc.dram_tensor(
        "wstack", [128 if packed else 65, HS], bf, kind="ExternalInput")
    d_wn = nc.dram_tensor("wn", [128, HS], bf, kind="ExternalInput")
    d_bnb = nc.dram_tensor("bnb", [128, L_CONV], f32, kind="ExternalInput")
    d_ab = nc.dram_tensor("ab", [128, L_CONV], f32, kind="ExternalInput")
    d_bb = nc.dram_tensor("bb", [128, L_CONV], f32, kind="ExternalInput")
    d_out = nc.dram_tensor("hsum", [128, 1], f32, kind="ExternalOutput")

    NPAIR = NT // 2
    GP = 4                      # pairs per DMA group (8 tiles)
    LAG = 3                     # scatter lags LAG pairs

    # window id per tile
    win_of = np.repeat(np.arange(NWIN), Tw)
    first_of = np.zeros(NT, bool)
    last_of = np.zeros(NT, bool)
    pos = 0
    for w in range(NWIN):
        first_of[pos] = True
        last_of[pos + int(Tw[w]) - 1] = True
        pos += int(Tw[w])

    with tile.TileContext(nc) as tc:
        with (
            tc.tile_pool(name="const", bufs=1) as constp,
            tc.tile_pool(name="ea", bufs=3) as eap,
            tc.tile_pool(name="ohp", bufs=4) as ohp,
            tc.tile_pool(name="ef", bufs=6) as efp,
            tc.tile_pool(name="swin", bufs=2) as swinp,
            tc.tile_pool(name="efps", bufs=5, space="PSUM") as efpsp,
            tc.tile_pool(name="sps", bufs=2, space="PSUM") as spsp,
            tc.tile_pool(name="big", bufs=1) as bigp,
        ):
            wstack_sb = constp.tile([128 if packed else 65, HS], bf)
            nc.sync.dma_start(wstack_sb[:], d_wstack[:])
            wemb_sb = constp.tile([93, 128], bf)
            nc.sync.dma_start(wemb_sb[:], d_wemb[:])
            wn_sb = constp.tile([128, HS], bf)
            nc.sync.dma_start(wn_sb[:], d_wn[:])
            bnb_sb = constp.tile([128, L_CONV], f32)
            nc.sync.dma_start(bnb_sb[:], d_bnb[:])
            ab_sb = constp.tile([128, L_CONV], f32)
            nc.sync.dma_start(ab_sb[:], d_ab[:])
            bb_sb = constp.tile([128, L_CONV], f32)
            nc.sync.dma_start(bb_sb[:], d_bb[:])
            ident_sb = constp.tile([128, 128], bf)
            make_identity(nc, ident_sb[:])

            xt_sb = bigp.tile([93, NPAD], bf)
            NK = NPAD // 512
            for k in range(NK):
                nc.sync.dma_start(
                    xt_sb[:, k * 512:(k + 1) * 512],
                    d_xt[:, k * 512:(k + 1) * 512])
            st_sb = bigp.tile([128, L_CONV * NPAD], bf)   # S^T per layer
            h_sb = bigp.tile([128, NPAD], bf)
            hs_sb = bigp.tile([128, NPAD], bf)
            t_sb = bigp.tile([128, NPAD], bf)
            u_sb = bigp.tile([128, NPAD], bf)
            hsum_sb = bigp.tile([128, 1], f32)

            node_steps = []

            def queue_node_chunk(k):
                """Embed + all 4 conv layers for node chunk k (512 nodes),
                as a list of small steps drip-fed one per pair so dependent
                chains never block the in-order PE FIFO.  Elementwise on
                GpSimd (idle engine), relu+bias on ScalarE, matmuls on
                TensorE.  Requires windows 4k..4k+3 transposed."""
                sl = slice(k * 512, (k + 1) * 512)

                def s_emb():
                    h_ps = spsp.tile(
                        [128, 512], f32, tag="nodeps", bufs=1, name="h_ps")
                    nc.tensor.matmul(
                        h_ps[:], wemb_sb[:], xt_sb[:, sl],
                        start=True, stop=True)
                    nc.scalar.copy(h_sb[:, sl], h_ps[:])
                node_steps.append(s_emb)

                def s_mul(l):
                    def f():
                        nc.gpsimd.tensor_tensor(
                            hs_sb[:, sl], h_sb[:, sl],
                            st_sb[:, l * NPAD + k * 512:
                                  l * NPAD + (k + 1) * 512], op=MULT)
                    return f

                def s_mm(l):
                    def f():
                        u_ps = spsp.tile(
                            [128, 512], f32, tag="nodeps", bufs=1,
                            name="u_ps")
                        nc.tensor.matmul(
                            u_ps[:], wn_sb[:, l * 128:(l + 1) * 128],
                            hs_sb[:, sl], start=True, stop=True)
                        nc.scalar.activation(
                            t_sb[:, sl], u_ps[:], Relu,
                            bias=bnb_sb[:, l:l + 1], scale=1.0)
                    return f

                def s_aff(l):
                    def f():
                        nc.gpsimd.tensor_scalar(
                            u_sb[:, sl], t_sb[:, sl], ab_sb[:, l:l + 1],
                            bb_sb[:, l:l + 1], op0=MULT, op1=ADD)
                        nc.gpsimd.tensor_tensor(
                            h_sb[:, sl], h_sb[:, sl], u_sb[:, sl], op=ADD)
                    return f

                for l in range(L_CONV):
                    node_steps.append(s_mul(l))
                    node_steps.append(None)   # spacing before dependent MM
                    node_steps.append(s_mm(l))
                    node_steps.append(s_aff(l))

            # ---------------- edge stage (software-pipelined) ------------
            state = {"s_ps": None}
            ef_tiles = [None] * NPAIR
            oh_groups = [None] * NPAIR

            def emit_scatter(t, ef_pair, oh_g, goff):
                w = int(win_of[t])
                if first_of[t]:
                    state["s_ps"] = spsp.tile(
                        [128, HS], f32, tag="sps", name="s_ps")
                s_ps = state["s_ps"]
                j = t - goff
                nc.tensor.matmul(
                    s_ps[:], oh_g[:, j * 128:(j + 1) * 128],
                    ef_pair[:, (t % 2) * HS:(t % 2 + 1) * HS],
                    start=bool(first_of[t]), stop=bool(last_of[t]))
                if last_of[t]:
                    s_sb = swinp.tile([128, HS], bf, tag="swin")
                    nc.scalar.copy(s_sb[:], s_ps[:])
                    for l in range(L_CONV):
                        tr_ps = spsp.tile([128, 128], bf, tag="sps")
                        nc.tensor.transpose(
                            tr_ps[:], s_sb[:, l * 128:(l + 1) * 128],
                            ident_sb[:])
                        dst = st_sb[:, l * NPAD + w * WIN:
                                    l * NPAD + (w + 1) * WIN]
                        nc.vector.tensor_copy(dst, tr_ps[:])
                    if (w + 1) % 4 == 0:
                        queue_node_chunk((w + 1) // 4 - 1)

            parity = 0
            ea_g = oh_g = None
            g0p = 0
            for p in range(NPAIR):
                if p % GP == 0:
                    gs = min(GP, NPAIR - p)
                    g0p = p
                    ea_g = eap.tile(
                        [128, GP * 128] if packed else [65, GP * 256], bf,
                        tag="ea")
                    if packed:
                        nc.sync.dma_start(
                            ea_g[:, :gs * 128],
                            d_eat[:, p * 128:(p + gs) * 128])
                    else:
                        nc.sync.dma_start(
                            ea_g[:, :gs * 256],
                            d_eat[:, p * 256:(p + gs) * 256])
                    oh_g = ohp.tile([128, GP * 256], bf, tag="oh")
                    nc.sync.dma_start(
                        oh_g[:, :gs * 256], d_oh[:, p * 256:(p + gs) * 256])

                jp = p - g0p
                ef_ps0 = efpsp.tile([128, HS], f32, tag="efps", name="ef_ps0")
                ef_ps1 = efpsp.tile([128, HS], f32, tag="efps", name="ef_ps1")
                if packed:
                    pe = ea_g[:, jp * 128:(jp + 1) * 128]
                    nc.tensor.matmul(
                        ef_ps0[0:64, :], pe[0:64, 0:64], wstack_sb[0:64, :],
                        start=True, stop=True, tile_position=(0, 0))
                    nc.tensor.matmul(
                        ef_ps0[64:128, :], pe[0:64, 64:128],
                        wstack_sb[0:64, :],
                        start=True, stop=True, tile_position=(0, 64))
                    nc.tensor.matmul(
                        ef_ps1[0:64, :], pe[64:128, 0:64],
                        wstack_sb[64:128, :],
                        start=True, stop=True, tile_position=(64, 0))
                    nc.tensor.matmul(
                        ef_ps1[64:128, :], pe[64:128, 64:128],
                        wstack_sb[64:128, :],
                        start=True, stop=True, tile_position=(64, 64))
                else:
                    nc.tensor.matmul(
                        ef_ps0[:], ea_g[:, jp * 256:jp * 256 + 128],
                        wstack_sb[:], start=True, stop=True)
                    nc.tensor.matmul(
                        ef_ps1[:],
                        ea_g[:, jp * 256 + 128:(jp + 1) * 256],
                        wstack_sb[:], start=True, stop=True)

                ef_pair = efp.tile([128, 2 * HS], bf, tag="ef")
                for half, ps in ((0, ef_ps0), (1, ef_ps1)):
                    dst = ef_pair[:, half * HS:(half + 1) * HS]
                    if parity:
                        nc.scalar.activation(dst, ps[:], Relu)
                    else:
                        nc.vector.tensor_scalar_max(dst, ps[:], 0.0)
                    parity ^= 1
                ef_tiles[p] = ef_pair
                oh_groups[p] = (oh_g, 2 * g0p)

                if p >= LAG:
                    q = p - LAG
                    og, goff = oh_groups[q]
                    emit_scatter(2 * q, ef_tiles[q], og, goff)
                    emit_scatter(2 * q + 1, ef_tiles[q], og, goff)
                    ef_tiles[q] = None
                if node_steps:
                    step = node_steps.pop(0)
                    if step is not None:
                        step()
            for q in range(max(NPAIR - LAG, 0), NPAIR):
                og, goff = oh_groups[q]
                emit_scatter(2 * q, ef_tiles[q], og, goff)
                emit_scatter(2 * q + 1, ef_tiles[q], og, goff)
            for step in node_steps:
                if step is not None:
                    step()

            nc.vector.tensor_reduce(
                hsum_sb[:], h_sb[:, :NPC], axis=mybir.AxisListType.X,
                op=ADD)
            nc.sync.dma_start(d_out[:], hsum_sb[:])

    nc.finalize()
    return nc


LAST_EXEC_NS = None


def kernel(x, edge_index, edge_attr, W_emb, b_emb, We, be, Wn, bn,
           g_c, beta_c, mu_c, var_c, Wd, bd, g_d, beta_d, mu_d, var_d, Wf, bf):
    global LAST_EXEC_NS

    packed = bool(np.all(np.asarray(be) == 0.0))
    prep = _host_prep(x, edge_index, edge_attr, W_emb, b_emb, We, be, packed)
    NT, Tw = prep["NT"], prep["Tw"]

    Wnf = np.asarray(Wn, np.float32)
    wn_stack = np.zeros((128, HS), np.float32)
    for l in range(L_CONV):
        wn_stack[:, l * H:(l + 1) * H] = Wnf[l]
    A = (np.asarray(g_c, np.float32)
         / np.sqrt(np.asarray(var_c, np.float32) + EPS))        # [L, H]
    B = np.asarray(beta_c, np.float32) - np.asarray(mu_c, np.float32) * A

    key = (NT, tuple(int(v) for v in Tw), packed)
    if key not in _cache:
        _cache[key] = _build_program(NT, Tw, packed)
    nc = _cache[key]

    common = {
        "wemb": prep["wemb"],
        "wstack": prep["wstack"],
        "wn": wn_stack.astype(BF16),
        "bnb": np.ascontiguousarray(np.asarray(bn, np.float32).T).reshape(128, L_CONV),
        "ab": np.ascontiguousarray(A.T).reshape(128, L_CONV),
        "bb": np.ascontiguousarray(B.T).reshape(128, L_CONV),
    }
    in_maps = []
    for c in range(NCORES):
        m = dict(common)
        m["eat"] = prep["eat"][c]
        m["oh"] = prep["oh"][c]
        m["xt"] = prep["xt"][c]
        in_maps.append(m)

    trace = bool(os.environ.get("KERNEL_TRACE"))
    if trace:
        try:
            from trn_agent_boot.trn_boot import _ntff_profile_via_ctypes
            from antenv.axon_hooks import set_axon_ntff_profile_hook
            set_axon_ntff_profile_hook(
                _ntff_profile_via_ctypes("/opt/axon/libaxon_pjrt.so"))
        except Exception:
            trace = False

    res = run_bass_kernel_spmd(
        nc, in_maps, core_ids=list(range(NCORES)), trace=trace)
    LAST_EXEC_NS = res.exec_time_ns

    total = np.zeros(128, np.float64)
    for c in range(NCORES):
        total += res.results[c]["hsum"].reshape(128).astype(np.float64)
    v = (total / N).astype(np.float32)

    # dense head on host (0.000001% of total FLOPs)
    g_df = np.asarray(g_d, np.float32)
    var_df = np.asarray(var_d, np.float32)
    beta_df = np.asarray(beta_d, np.float32)
    mu_df = np.asarray(mu_d, np.float32)
    Wdf = np.asarray(Wd, np.float32)
    bdf = np.asarray(bd, np.float32)
    for d in range(L_DENSE):
        v = np.maximum(v @ Wdf[d] + bdf[d], 0.0)
        Ad = g_df[d] / np.sqrt(var_df[d] + EPS)
        v = (v - mu_df[d]) * Ad + beta_df[d]
    out = v @ np.asarray(Wf, np.float32) + np.asarray(bf, np.float32)
    return out.astype(np.float32)
